# revision 1
# baseline (speedup 1.0000x reference)
"""EdgeConv-style GNN message passing kernel for Trainium2 (Bass/Tile).

Computes, for each edge e = (s, d):
    proj = x @ w1.T + b1                      # [N, H]  (node projection)
    h_e  = relu(proj[s] + proj[d])            # [E, H]
    out_e = [h_e | edge_attr_e | edge_f_e] @ w2.T + b2   # [E, O]

Sharding: edges are split evenly across 8 NeuronCores; x and the small
linear weights are replicated.  Each core computes the full proj table
locally, stores it in a DRAM scratch buffer, then gathers the two
endpoint rows per edge with the batched SWDGE gather (InstDMAGatherAnt).

That gather takes int16 indices (max 32767), so nodes are addressed with
a stride-4 trick: gather pass r reads rows at base offset r rows with row
stride 4 rows (1024B), index = node>>2 (<= 25087).  Edges are bucketed on
the host by (src&3, dst&3) into 16 blocks of 512 slots per 8192-slot
macro; the host permutes edge_attr/edge_f into that slot order and
inverse-permutes the output rows during unshard.  Only the low 2 bits of
the node ids drive the bucketing, so the gather stays random-access.

g_all row layout per macro, in units of 128 rows (gathered 16384 rows):
    [sr0(16u) | ds0(16u) | sr1 | ds1 | sr2 | ds2 | sr3 | ds3]
  - gather call r writes units [r*32, (r+1)*32)
  - src rows of block (r,s) at unit  r*32 + s*4      (4 units = 512 rows)
  - dst rows of block (r,s) at unit  s*32 + 16 + r*4
  - hs (edge slot) unit of block (r,s) = r*16 + s*4
"""

import math

import numpy as np

import concourse.bacc as bacc
import concourse.bass as bass
import concourse.mybir as mybir
from concourse import library_config
from concourse.bass_utils import run_bass_kernel_spmd
from concourse.masks import make_identity
from concourse.tile import TileContext, add_dep_helper

F32 = mybir.dt.float32
I16 = mybir.dt.int16
RELU = mybir.ActivationFunctionType.Relu

N_CORES = 8
NF = 64   # node feature dim (lin1 input)
NH = 64   # hidden dim (lin1 output)
EA = 16   # edge_attr dim
EF = 16   # edge_f dim
CF = NH + EA + EF  # concat feature dim = 96
OD = 64   # output dim

NODE_MACRO = 1024         # nodes per phase-1 macro tile (8 blocks of 128)
BLK = 512                 # edges per (r,s) bucket block
MACRO = 16 * BLK          # 8192 edge slots per phase-2 macro
N_GROUPS = MACRO // 512   # 16 groups of 512 edge slots per macro

TRACE = False
LAST_RESULTS = None


def _build_nc(
    n_pad: int, nm_edge: int, b1_nz: bool, b2_nz: bool, p2_only: bool = False
) -> bass.Bass:
    assert n_pad % NODE_MACRO == 0
    nm_node = n_pad // NODE_MACRO
    e_slots = nm_edge * MACRO

    nc = bacc.Bacc()
    x = nc.declare_dram_parameter("x", [n_pad, NF], F32, isOutput=False)
    w1t = nc.declare_dram_parameter("w1t", [128, NH], F32, isOutput=False)
    w2t = nc.declare_dram_parameter("w2t", [CF, OD], F32, isOutput=False)
    # per macro: 4 gather calls x 4096 int16 idx, each wrapped [128, 256]
    idx = nc.declare_dram_parameter("idx", [nm_edge, 128, 1024], I16, isOutput=False)
    ea = nc.declare_dram_parameter("ea", [e_slots, EA], F32, isOutput=False)
    ef = nc.declare_dram_parameter("ef", [e_slots, EF], F32, isOutput=False)
    if b1_nz:
        b1r = nc.declare_dram_parameter("b1r", [1, 512], F32, isOutput=False)
    if b2_nz:
        b2r = nc.declare_dram_parameter("b2r", [128, 256], F32, isOutput=False)
    out = nc.declare_dram_parameter("out", [e_slots, OD], F32, isOutput=True)
    if p2_only:
        proj = nc.declare_dram_parameter("proj", [n_pad, NH], F32, isOutput=False)
    else:
        proj = nc.dram_tensor("proj", [n_pad, NH], F32)
    proj4 = proj[:, :].rearrange("(q r) f -> q r f", r=4)

    with TileContext(nc) as tc:
        with tc.tile_pool(name="const", bufs=1) as cpool:
            libload = nc.gpsimd.load_library(library_config.mlp)
            ident = cpool.tile([128, 128], F32)
            make_identity(nc, ident[:])
            w1t_sb = cpool.tile([128, NH], F32)
            nc.sync.dma_start(out=w1t_sb[:], in_=w1t[:])
            w2t_sb = cpool.tile([CF, OD], F32)
            nc.sync.dma_start(out=w2t_sb[:], in_=w2t[:])
            if b1_nz:
                b1r_sb = cpool.tile([1, 512], F32)
                nc.sync.dma_start(out=b1r_sb[:], in_=b1r[:])
                ones_sb = cpool.tile([1, 128], F32)
                nc.gpsimd.memset(ones_sb[:], 1.0)
            if b2_nz:
                b2r_sb = cpool.tile([128, 256], F32)
                nc.sync.dma_start(out=b2r_sb[:], in_=b2r[:])

            # ---------------- phase 1: proj = x @ w1.T (+ b1) ----------------
            proj_stores = []
            with (
                tc.tile_pool(name="p1", bufs=3) as p1pool,
                tc.tile_pool(name="p1psA", bufs=2, space="PSUM") as ps_xt_pool,
                tc.tile_pool(name="p1psB", bufs=2, space="PSUM") as ps_pr_pool,
            ):
                for m in range(0 if p2_only else nm_node):
                    x_t = p1pool.tile([128, 8, NF], F32, tag="x")
                    nc.sync.dma_start(
                        out=x_t[:],
                        in_=x[m * 1024:(m + 1) * 1024].rearrange(
                            "(t p) f -> p t f", p=128
                        ),
                    )
                    # single-block transposes: everything stays at SBUF/PSUM
                    # partition 0 (partition-offset matmul operands crash HW)
                    xT_sb = p1pool.tile([64, 1024], F32, tag="xT")
                    for half in range(2):
                        ps_xT = ps_xt_pool.tile([64, 512], F32, tag="psxT")
                        for t4 in range(4):
                            t = half * 4 + t4
                            nc.tensor.transpose(
                                out=ps_xT[:, t4 * 128:(t4 + 1) * 128],
                                in_=x_t[:, t, :],
                                identity=ident[:],
                            )
                        nc.vector.tensor_copy(
                            out=xT_sb[:, half * 512:(half + 1) * 512],
                            in_=ps_xT[:],
                        )
                    ps_proj = ps_pr_pool.tile([128, 512], F32, tag="psproj")
                    if b1_nz:
                        nc.tensor.matmul(
                            out=ps_proj[:],
                            lhsT=ones_sb[:1, :],
                            rhs=b1r_sb[:1, :],
                            start=True,
                            stop=False,
                            skip_group_check=True,
                        )
                    for t in range(8):
                        nc.tensor.matmul(
                            out=ps_proj[:, t * 64:(t + 1) * 64],
                            lhsT=xT_sb[:, t * 128:(t + 1) * 128],
                            rhs=w1t_sb[:64, :],
                            start=not b1_nz,
                            stop=(t == 7) if b1_nz else True,
                            skip_group_check=b1_nz,
                        )
                    proj_sb = p1pool.tile([128, 512], F32, tag="proj")
                    nc.scalar.copy(out=proj_sb[:], in_=ps_proj[:])
                    st = nc.scalar.dma_start(
                        out=proj[m * 1024:(m + 1) * 1024].rearrange(
                            "(t p) f -> p t f", p=128
                        ),
                        in_=proj_sb[:].rearrange("p (t f) -> p t f", t=8),
                    )
                    proj_stores.append(st)

            join = None
            if not p2_only:
                join = nc.sync.nop(nofuse=True, hint="proj_done_join")
                for st in proj_stores:
                    add_dep_helper(
                        join.ins, st.ins, reason="join waits on proj store"
                    )

            # ---------------- phase 2: per-edge compute ----------------
            with (
                tc.tile_pool(name="p2idx", bufs=2) as idxpool,
                tc.tile_pool(name="p2g", bufs=2) as gpool,
                tc.tile_pool(name="p2hs", bufs=2) as hspool,
                tc.tile_pool(name="p2af", bufs=2) as afpool,
                tc.tile_pool(name="p2ft", bufs=4) as ftpool,
                tc.tile_pool(name="p2o", bufs=3) as opool,
                tc.tile_pool(name="p2psF", bufs=3, space="PSUM") as psf_pool,
                tc.tile_pool(name="p2psO", bufs=3, space="PSUM") as pso_pool,
            ):
                for m in range(nm_edge):
                    idx_t = idxpool.tile([128, 1024], I16, tag="idx")
                    nc.gpsimd.dma_start(out=idx_t[:], in_=idx[m])
                    g = gpool.tile([128, 128, NH], F32, tag="g")
                    for r in range(4):
                        gi = nc.gpsimd.dma_gather(
                            out_ap=g[:, r * 32:(r + 1) * 32, :],
                            in_ap=proj4[:, r, :],
                            idxs_ap=idx_t[:, r * 256:(r + 1) * 256],
                            num_idxs=4096,
                            num_idxs_reg=4096,
                            elem_size=NH,
                            elem_step=4 * NH,
                            single_packet=False,
                        )
                        add_dep_helper(
                            gi.ins, libload.ins, reason="gather after lib load"
                        )
                        if join is not None:
                            add_dep_helper(
                                gi.ins, join.ins, reason="gather waits on proj"
                            )
                    hs = hspool.tile([128, 64, NH], F32, tag="hs")
                    # per-(r,s)-block adds: each depends on only 2 gather
                    # calls, so they overlap the remaining gathers
                    for r in range(4):
                        for s in range(4):
                            su = r * 32 + s * 4
                            du = s * 32 + 16 + r * 4
                            hu = r * 16 + s * 4
                            nc.vector.tensor_add(
                                out=hs[:, hu:hu + 4, :],
                                in0=g[:, su:su + 4, :],
                                in1=g[:, du:du + 4, :],
                            )
                    asm = afpool.tile([128, 64, CF], F32, tag="asm")
                    nc.scalar.activation(
                        out=asm[:, :, 0:NH], in_=hs[:], func=RELU
                    )
                    base_e = m * MACRO
                    nc.gpsimd.dma_start(
                        out=asm[:, :, NH:NH + EA],
                        in_=ea[base_e:base_e + MACRO].rearrange(
                            "(j p) f -> p j f", p=128
                        ),
                    )
                    nc.gpsimd.dma_start(
                        out=asm[:, :, NH + EA:CF],
                        in_=ef[base_e:base_e + MACRO].rearrange(
                            "(j p) f -> p j f", p=128
                        ),
                    )
                    for grp in range(N_GROUPS):
                        ps_f = psf_pool.tile([CF, 512], F32, tag="psf")
                        for j4 in range(4):
                            j = grp * 4 + j4
                            nc.tensor.transpose(
                                out=ps_f[:, j4 * 128:(j4 + 1) * 128],
                                in_=asm[:, j, :],
                                identity=ident[:],
                            )
                        fT = ftpool.tile([CF, 512], F32, tag="ft")
                        if grp % 2 == 0:
                            nc.vector.tensor_copy(out=fT[:], in_=ps_f[:])
                        else:
                            nc.scalar.copy(out=fT[:], in_=ps_f[:])
                        ps_o = pso_pool.tile([128, 256], F32, tag="pso")
                        for j4 in range(4):
                            nc.tensor.matmul(
                                out=ps_o[:, j4 * 64:(j4 + 1) * 64],
                                lhsT=fT[:, j4 * 128:(j4 + 1) * 128],
                                rhs=w2t_sb[:],
                                start=True,
                                stop=True,
                            )
                        if grp % 8 == 0:
                            o_big = opool.tile([128, 8, 256], F32, tag="o")
                        o_sb = o_big[:, grp % 8, :]
                        if b2_nz:
                            nc.vector.tensor_add(
                                out=o_sb, in0=ps_o[:], in1=b2r_sb[:]
                            )
                        elif grp % 2 == 0:
                            nc.scalar.copy(out=o_sb, in_=ps_o[:])
                        else:
                            nc.vector.tensor_copy(out=o_sb, in_=ps_o[:])
                        if grp % 8 == 7:
                            base = base_e + (grp - 7) * 512
                            nc.sync.dma_start(
                                out=out[base:base + 4096].rearrange(
                                    "(g j p) f -> p (g j) f", p=128, j=4
                                ),
                                in_=o_big[:].rearrange("p g (j f) -> p (g j) f", j=4),
                            )
    nc.compile()
    return nc


def _shard_core(src, dst, nm_edge):
    """Bucket one core's edges by (src&3, dst&3) into the macro/block layout.

    Returns (pos, idx16) where pos[e] is the edge's slot index in
    [0, nm_edge*MACRO) and idx16 is the [nm_edge, 128, 1024] int16 gather
    index tensor.
    """
    n = src.shape[0]
    key = ((src & 3) << 2 | (dst & 3)).astype(np.int8)
    order = np.argsort(key, kind="stable")
    sorted_key = key[order]
    # rank of each sorted element within its bucket
    bstart = np.searchsorted(sorted_key, np.arange(16))
    wb = np.arange(n) - bstart[sorted_key]
    r = (sorted_key >> 2).astype(np.int64)
    s = (sorted_key & 3).astype(np.int64)
    chunk = wb // BLK
    off = wb % BLK
    slot_sorted = chunk * MACRO + (r * 16 + s * 4) * 128 + off
    pos = np.empty(n, dtype=np.int64)
    pos[order] = slot_sorted

    # gather index arrays: SRCV[m, r, s, off] / DSTV[m, s, r, off]
    srcv = np.zeros((nm_edge, 4, 4, BLK), dtype=np.int16)
    dstv = np.zeros((nm_edge, 4, 4, BLK), dtype=np.int16)
    srcq = (src[order] >> 2).astype(np.int16)
    dstq = (dst[order] >> 2).astype(np.int16)
    srcv[chunk, r, s, off] = srcq
    dstv[chunk, s, r, off] = dstq

    idx16 = np.zeros((nm_edge, 128, 1024), dtype=np.int16)
    for rr in range(4):
        # call rr list: [src blocks (rr, 0..3) | dst blocks (0..3, rr)],
        # 4096 idxs, wrapped as [16, 256] then replicated to 128 partitions
        lst = np.concatenate(
            [srcv[:, rr].reshape(nm_edge, 2048),
             dstv[:, rr].reshape(nm_edge, 2048)],
            axis=1,
        )  # [nm, 4096]
        wrapped = lst.reshape(nm_edge, 256, 16).transpose(0, 2, 1)  # [nm,16,256]
        idx16[:, :, rr * 256:(rr + 1) * 256] = np.tile(wrapped, (1, 8, 1))
    return pos, idx16


def prepare(x, edge_index, edge_attr, edge_f, w1, b1, w2, b2):
    """Build the Bass program + per-core input maps. Returns (nc, in_maps, meta)."""
    x = np.asarray(x, dtype=np.float32)
    edge_index = np.asarray(edge_index)
    edge_attr = np.asarray(edge_attr, dtype=np.float32)
    edge_f = np.asarray(edge_f, dtype=np.float32)
    w1 = np.asarray(w1, dtype=np.float32)
    b1 = np.asarray(b1, dtype=np.float32)
    w2 = np.asarray(w2, dtype=np.float32)
    b2 = np.asarray(b2, dtype=np.float32)

    n_nodes = x.shape[0]
    n_edges = edge_index.shape[1]
    e_pc = math.ceil(n_edges / N_CORES)
    n_pad = math.ceil(n_nodes / NODE_MACRO) * NODE_MACRO

    b1_nz = bool(np.any(b1))
    b2_nz = bool(np.any(b2))

    ei = edge_index.astype(np.int64)
    cores = []
    nm_edge = 1
    for c in range(N_CORES):
        lo = c * e_pc
        hi = min(lo + e_pc, n_edges)
        src = ei[0, lo:hi]
        dst = ei[1, lo:hi]
        key = (src & 3) * 4 + (dst & 3)
        counts = np.bincount(key, minlength=16)
        nm_edge = max(nm_edge, int(math.ceil(counts.max() / BLK)))
        cores.append((lo, hi, src, dst))

    nc = _build_nc(n_pad, nm_edge, b1_nz, b2_nz)
    e_slots = nm_edge * MACRO

    x_pad = x if n_pad == n_nodes else np.concatenate(
        [x, np.zeros((n_pad - n_nodes, NF), np.float32)], axis=0
    )
    w1t_rep = np.ascontiguousarray(np.tile(w1.T, (2, 1)))          # [128, NH]
    w2t = np.ascontiguousarray(w2.T)                               # [CF, OD]
    b1r = np.ascontiguousarray(np.tile(b1, 8)[None, :])            # [1, 512]
    b2r = np.ascontiguousarray(np.tile(b2, (128, 4)))              # [128, 256]

    in_maps = []
    positions = []
    for c in range(N_CORES):
        lo, hi, src, dst = cores[c]
        pos, idx16 = _shard_core(src, dst, nm_edge)
        positions.append(pos)
        ea_c = np.zeros((e_slots, EA), np.float32)
        ea_c[pos] = edge_attr[lo:hi]
        ef_c = np.zeros((e_slots, EF), np.float32)
        ef_c[pos] = edge_f[lo:hi]
        m = {
            "x": x_pad,
            "w1t": w1t_rep,
            "w2t": w2t,
            "idx": idx16,
            "ea": ea_c,
            "ef": ef_c,
        }
        if b1_nz:
            m["b1r"] = b1r
        if b2_nz:
            m["b2r"] = b2r
        in_maps.append(m)

    meta = {"e_pc": e_pc, "n_edges": n_edges, "positions": positions}
    return nc, in_maps, meta


def kernel(x, edge_index, edge_attr, edge_f, w1, b1, w2, b2):
    global LAST_RESULTS
    nc, in_maps, meta = prepare(
        x, edge_index, edge_attr, edge_f, w1, b1, w2, b2
    )
    res = run_bass_kernel_spmd(nc, in_maps, list(range(N_CORES)), trace=TRACE)
    LAST_RESULTS = res

    e_pc, n_edges = meta["e_pc"], meta["n_edges"]
    parts = []
    for c in range(N_CORES):
        parts.append(res.results[c]["out"][meta["positions"][c]])
    return np.ascontiguousarray(np.concatenate(parts, axis=0), dtype=np.float32)



# revision 5
# speedup vs baseline: 1.3349x; 1.3349x over previous
"""EdgeConv-style GNN message passing kernel for Trainium2 (Bass/Tile).

Computes, for each edge e = (s, d):
    proj = x @ w1.T + b1                      # [N, H]  (node projection)
    h_e  = relu(proj[s] + proj[d])            # [E, H]
    out_e = [h_e | edge_attr_e | edge_f_e] @ w2.T + b2   # [E, O]

Design (bf16 throughout, fp32 PSUM accumulation):
- Edges are sharded across 8 cores; x and weights are replicated.
- Phase 1 builds the full bf16 proj table in SBUF (no DRAM round trip):
  host-pretransposed x columns stream in, 8 matmuls per 1024 nodes write
  PSUM, one vector copy lands the quad-packed table slice. No on-chip
  transposes anywhere (host pre-permutes all layouts).
- Phase 2 uses the SBUF-source transpose-mode SWDGE gather: the table
  packs node quads (4 rows = 512B stripes, token = node>>2 <= 25087 so
  int16 indices fit); each 256B gathered element covers 2 adjacent rows
  and lands FEATURE-MAJOR across partitions: the addressed node's row at
  partitions [0:64) or [64:128) depending on the per-call
  sbuf_byte_offset = off*128, off in {0,1,2}.
- Edges are host-sorted into groups keyed by (src_off, dst_off, form):
  form A: both rows at slab0; form B: both at slab64 (DVE tensor ops
  need equal input base partitions); forms C0/C1 (src&3==0 with
  dst&3==3, or vice versa, 12.5% of edges) use one extra aligned copy.
  Group sizes are padded to the max across cores so one SPMD program
  fits all 8 cores; gather call boundaries and num_idxs are baked.
- feat.T tiles ([96, W]: relu(h).T on partitions 0..63, host-transposed
  [edge_attr|edge_f].T streamed into 64..95) feed weight-stationary
  matmuls; out is written transposed ([64, E_slots] bf16) and the host
  inverts layout/permutation and converts to fp32 (host prep is free).
"""

import math

import numpy as np

import concourse.bacc as bacc
import concourse.bass as bass
import concourse.mybir as mybir
from concourse import library_config
from concourse.bass_utils import run_bass_kernel_spmd
from concourse.tile import TileContext, add_dep_helper

F32 = mybir.dt.float32
BF16 = mybir.dt.bfloat16
I16 = mybir.dt.int16
RELU = mybir.ActivationFunctionType.Relu

N_CORES = 8
NF = 64   # node feature dim
NH = 64   # hidden dim
EA = 16   # edge_attr dim
EF = 16   # edge_f dim
CF = NH + EA + EF  # concat feature dim = 96
OD = 64   # output dim

TPR = 128            # gather tokens (quads) per partition round
QBYTES = 512         # bytes per quad stripe (4 rows x 128B)
RANKS = 196          # quad ranks: 25088 quads = 100352 node slots
NQ = TPR * RANKS
N_PAD = 4 * NQ       # padded node count (100352)
P1_ITERS = RANKS // 2  # phase-1 iterations (2 ranks = 1024 nodes each)
W = 4096             # edge slots per phase-2 macro

# group table: (src_off, dst_off, form), (so,do)-major so gather calls
# merge across adjacent groups. form 0=A(slab0) 1=B(slab64) 2=C0 3=C1.
GROUPS = []
for _so in range(3):
    for _do in range(3):
        GROUPS.append((_so, _do, 0))
        GROUPS.append((_so, _do, 1))
        if (_so, _do) == (0, 2):
            GROUPS.append((_so, _do, 2))
        if (_so, _do) == (2, 0):
            GROUPS.append((_so, _do, 3))
GID = {g: i for i, g in enumerate(GROUPS)}
NG = len(GROUPS)

TRACE = False
LAST_RESULTS = None


def _edge_groups(src, dst):
    """Per-edge (group id, src token, dst token)."""
    s3 = (src & 3).astype(np.int64)
    d3 = (dst & 3).astype(np.int64)
    is_a = (s3 < 3) & (d3 < 3)
    is_c0 = (s3 == 0) & (d3 == 3)
    is_c1 = (s3 == 3) & (d3 == 0)
    form = np.where(is_a, 0, np.where(is_c0, 2, np.where(is_c1, 3, 1)))
    so = np.where(form == 0, s3, 0)
    so = np.where(form == 1, s3 - 1, so)
    so = np.where(form == 2, 0, so)
    so = np.where(form == 3, 2, so)
    do = np.where(form == 0, d3, 0)
    do = np.where(form == 1, d3 - 1, do)
    do = np.where(form == 2, 2, do)
    do = np.where(form == 3, 0, do)
    lut = np.full((3, 3, 4), -1, dtype=np.int64)
    for (a, b, f), i in GID.items():
        lut[a, b, f] = i
    gid = lut[so, do, form]
    assert (gid >= 0).all()
    return gid, (src >> 2).astype(np.int16), (dst >> 2).astype(np.int16)


def _macro_specs(m_sizes):
    """Shared-across-cores macro segmentation from padded group sizes."""
    starts = np.concatenate([[0], np.cumsum(m_sizes)])
    e_slots = int(starts[-1])
    nm = math.ceil(e_slots / W)
    specs = []
    for m in range(nm):
        mo = m * W
        lm = min(W, e_slots - mo)
        segs = []
        for g, (so, do, form) in enumerate(GROUPS):
            a = max(int(starts[g]), mo)
            b = min(int(starts[g + 1]), mo + lm)
            if a < b:
                segs.append((so, do, form, a - mo, b - mo))
        src_calls, dst_calls = [], []
        for so, do, form, a, b in segs:
            if src_calls and src_calls[-1][0] == so and src_calls[-1][2] == a:
                src_calls[-1] = (so, src_calls[-1][1], b)
            else:
                src_calls.append((so, a, b))
            if dst_calls and dst_calls[-1][0] == do and dst_calls[-1][2] == a:
                dst_calls[-1] = (do, dst_calls[-1][1], b)
            else:
                dst_calls.append((do, a, b))
        adds = [(form, a, b) for so, do, form, a, b in segs]
        specs.append({
            "off": mo, "L": lm,
            "src_calls": src_calls,
            "dst_calls": dst_calls,
            "adds": adds,
        })
    return specs, e_slots, nm


def _build_nc(macros, e_slots, nm, kin, cfp, b2_nz):
    nc = bacc.Bacc(num_swdge_queues=4)
    xg = nc.declare_dram_parameter("xg", [kin, N_PAD], BF16, isOutput=False)
    w1t = nc.declare_dram_parameter("w1t", [kin, NH], BF16, isOutput=False)
    w2t = nc.declare_dram_parameter("w2t", [cfp, OD], BF16, isOutput=False)
    idxd = nc.declare_dram_parameter("idx", [nm, 128, W // 8], I16, isOutput=False)
    eaefd = nc.declare_dram_parameter("eaef", [2 * EA, e_slots], BF16, isOutput=False)
    outd = nc.declare_dram_parameter("outT", [OD, e_slots], BF16, isOutput=True)

    with TileContext(nc) as tc:
        with tc.tile_pool(name="const", bufs=1) as cpool:
            libload = nc.gpsimd.load_library(library_config.mlp)
            w1t_sb = cpool.tile([kin, NH], BF16)
            w1ld = nc.sync.dma_start(out=w1t_sb[:], in_=w1t[:])
            w2t_sb = cpool.tile([cfp, OD], BF16)
            w2ld = nc.sync.dma_start(out=w2t_sb[:], in_=w2t[:])
            table = cpool.tile([128, RANKS * 256], BF16)

            # ---------------- phase 1: proj table in SBUF ----------------
            tstores = []
            with (
                tc.tile_pool(name="p1", bufs=3) as p1pool,
                tc.tile_pool(name="p1ps", bufs=3, space="PSUM") as ps1pool,
            ):
                for i in range(P1_ITERS):
                    xt = p1pool.tile([kin, 1024], BF16, tag="x")
                    xld = nc.sync.dma_start(
                        out=xt[:], in_=xg[:, i * 1024:(i + 1) * 1024]
                    )
                    ps = ps1pool.tile([128, 512], F32, tag="ps")
                    for k in range(8):
                        mm = nc.tensor.matmul(
                            out=ps[:, k * 64:(k + 1) * 64],
                            lhsT=xt[:, k * 128:(k + 1) * 128],
                            rhs=w1t_sb[:],
                            start=True,
                            stop=True,
                        )
                        add_dep_helper(mm.ins, xld.ins, reason="mm after x load")
                        add_dep_helper(mm.ins, w1ld.ins, reason="mm after w1")
                    cp = nc.vector.tensor_copy(
                        out=table[:, i * 512:(i + 1) * 512], in_=ps[:]
                    )
                    tstores.append(cp)

            join = nc.sync.nop(nofuse=True, hint="table_done_join")
            for cp in tstores:
                add_dep_helper(join.ins, cp.ins, reason="join waits on table")

            # ---------------- phase 2: per-edge compute ----------------
            with (
                tc.tile_pool(name="p2idx", bufs=2) as idxpool,
                tc.tile_pool(name="p2gs", bufs=2) as gspool,
                tc.tile_pool(name="p2gd", bufs=2) as gdpool,
                tc.tile_pool(name="p2h", bufs=2) as hpool,
                tc.tile_pool(name="p2t", bufs=2) as tpool,
                tc.tile_pool(name="p2f", bufs=2) as fpool,
                tc.tile_pool(name="p2o", bufs=2) as opool,
                tc.tile_pool(name="p2ps", bufs=4, space="PSUM") as pspool,
            ):
                qn = 0
                for m, spec in enumerate(macros):
                    lm = spec["L"]
                    mo = spec["off"]
                    idx_t = idxpool.tile([128, W // 8], I16, tag="idx")
                    ixld = nc.scalar.dma_start(out=idx_t[:], in_=idxd[m])
                    gs = gspool.tile([128, 1, W], BF16, tag="gs")
                    gd = gdpool.tile([128, 1, W], BF16, tag="gd")
                    gcalls = []
                    for ep, calls, gt in (
                        (0, spec["src_calls"], gs),
                        (1, spec["dst_calls"], gd),
                    ):
                        base16 = (lm // 16) * ep
                        for off, a, b in calls:
                            gi = nc.gpsimd.dma_gather(
                                out_ap=gt[:, :, a:b],
                                in_ap=table[:, :],
                                idxs_ap=idx_t[
                                    :, base16 + a // 16:base16 + b // 16
                                ],
                                num_idxs=b - a,
                                num_idxs_reg=b - a,
                                elem_size=128,
                                transpose=True,
                                single_packet=False,
                                queue_num=0,
                                sbuf_tokens_per_rank=TPR,
                                sbuf_free_dim_per_rank=QBYTES,
                                sbuf_byte_offset=off * 128,
                            )
                            qn += 1
                            add_dep_helper(
                                gi.ins, libload.ins, reason="after lib load"
                            )
                            add_dep_helper(
                                gi.ins, join.ins, reason="gather after table"
                            )
                            add_dep_helper(
                                gi.ins, ixld.ins, reason="gather after idx"
                            )
                            gcalls.append(gi)
                    hp = hpool.tile([64, W], BF16, tag="h")
                    tmp = tpool.tile([64, W], BF16, tag="t")
                    addis = []
                    for form, a, b in spec["adds"]:
                        if form == 0:
                            ai = nc.vector.tensor_add(
                                out=hp[:, a:b],
                                in0=gs[0:64, 0, a:b],
                                in1=gd[0:64, 0, a:b],
                            )
                        elif form == 1:
                            ai = nc.vector.tensor_add(
                                out=hp[:, a:b],
                                in0=gs[64:128, 0, a:b],
                                in1=gd[64:128, 0, a:b],
                            )
                        elif form == 2:
                            ci = nc.vector.tensor_copy(
                                out=tmp[:, a:b], in_=gd[64:128, 0, a:b]
                            )
                            for gi in gcalls:
                                add_dep_helper(ci.ins, gi.ins, reason="g")
                            ai = nc.vector.tensor_add(
                                out=hp[:, a:b],
                                in0=gs[0:64, 0, a:b],
                                in1=tmp[:, a:b],
                            )
                        else:
                            ci = nc.vector.tensor_copy(
                                out=tmp[:, a:b], in_=gs[64:128, 0, a:b]
                            )
                            for gi in gcalls:
                                add_dep_helper(ci.ins, gi.ins, reason="g")
                            ai = nc.vector.tensor_add(
                                out=hp[:, a:b],
                                in0=gd[0:64, 0, a:b],
                                in1=tmp[:, a:b],
                            )
                        for gi in gcalls:
                            add_dep_helper(ai.ins, gi.ins, reason="g")
                        addis.append(ai)
                    ft = fpool.tile([cfp, W], BF16, tag="f")
                    if m % 2 == 0:
                        ri = nc.scalar.activation(
                            out=ft[0:64, 0:lm], in_=hp[:, 0:lm], func=RELU
                        )
                    else:
                        ri = nc.vector.tensor_scalar_max(
                            ft[0:64, 0:lm], hp[:, 0:lm], 0.0
                        )
                    for ai in addis:
                        add_dep_helper(ri.ins, ai.ins, reason="relu after add")
                    ld = nc.sync.dma_start(
                        out=ft[64:64 + 2 * EA, 0:lm],
                        in_=eaefd[:, mo:mo + lm],
                    )
                    exdeps = [ri, ld]
                    if b2_nz:
                        ms = nc.vector.memset(ft[CF:CF + 1, 0:lm], 1.0)
                        exdeps.append(ms)
                    ot = opool.tile([OD, W], BF16, tag="o")
                    for c in range(math.ceil(lm / 512)):
                        cw = min(512, lm - c * 512)
                        ps = pspool.tile([OD, 512], F32, tag="ps")
                        mi = nc.tensor.matmul(
                            out=ps[:, 0:cw],
                            lhsT=w2t_sb[:],
                            rhs=ft[:, c * 512:c * 512 + cw],
                            start=True,
                            stop=True,
                        )
                        for d in exdeps:
                            add_dep_helper(mi.ins, d.ins, reason="mm after ft")
                        add_dep_helper(mi.ins, w2ld.ins, reason="mm after w2")
                        if c % 2 == 0:
                            nc.scalar.copy(
                                out=ot[:, c * 512:c * 512 + cw], in_=ps[:, 0:cw]
                            )
                        else:
                            nc.vector.tensor_copy(
                                out=ot[:, c * 512:c * 512 + cw], in_=ps[:, 0:cw]
                            )
                    nc.sync.dma_start(
                        out=outd[:, mo:mo + lm], in_=ot[:, 0:lm]
                    )
    nc.compile()
    return nc


def prepare(x, edge_index, edge_attr, edge_f, w1, b1, w2, b2):
    """Build the Bass program + per-core input maps. Returns (nc, in_maps, meta)."""
    np_bf16 = mybir.dt.np(BF16)
    x = np.asarray(x, dtype=np.float32)
    edge_index = np.asarray(edge_index).astype(np.int64)
    edge_attr = np.asarray(edge_attr, dtype=np.float32)
    edge_f = np.asarray(edge_f, dtype=np.float32)
    w1 = np.asarray(w1, dtype=np.float32)
    b1 = np.asarray(b1, dtype=np.float32)
    w2 = np.asarray(w2, dtype=np.float32)
    b2 = np.asarray(b2, dtype=np.float32)

    n_nodes = x.shape[0]
    n_edges = edge_index.shape[1]
    assert n_nodes <= N_PAD
    e_pc = math.ceil(n_edges / N_CORES)
    b1_nz = bool(np.any(b1))
    b2_nz = bool(np.any(b2))
    kin = NF + 1 if b1_nz else NF
    cfp = CF + 1 if b2_nz else CF

    # per-core edge groups
    cores = []
    counts = np.zeros((N_CORES, NG), dtype=np.int64)
    for c in range(N_CORES):
        lo = c * e_pc
        hi = min(lo + e_pc, n_edges)
        src = edge_index[0, lo:hi]
        dst = edge_index[1, lo:hi]
        gid, stok, dtok = _edge_groups(src, dst)
        counts[c] = np.bincount(gid, minlength=NG)
        cores.append((lo, hi, gid, stok, dtok))
    m_sizes = (np.ceil(counts.max(axis=0) / 128) * 128).astype(np.int64)
    macros, e_slots, nm = _macro_specs(m_sizes)
    starts = np.concatenate([[0], np.cumsum(m_sizes)])

    nc = _build_nc(macros, e_slots, nm, kin, cfp, b2_nz)

    # ---- shared host tensors ----
    xpad = np.zeros((N_PAD, NF), dtype=np.float32)
    xpad[:n_nodes] = x
    j = np.arange(P1_ITERS * 1024)
    ji, jk, jp = j >> 10, (j >> 7) & 7, j & 127
    perm = 4 * ((2 * ji + (jk >> 2)) * 128 + jp) + (jk & 3)
    xgT = np.ascontiguousarray(xpad.T[:, perm]).astype(np_bf16)
    if b1_nz:
        xgT = np.concatenate(
            [xgT, np.ones((1, N_PAD), dtype=np_bf16)], axis=0
        )
    w1t_h = w1.T.astype(np_bf16)
    if b1_nz:
        w1t_h = np.concatenate([w1t_h, b1[None, :].astype(np_bf16)], axis=0)
    w2t_h = w2.T.astype(np_bf16)
    if b2_nz:
        w2t_h = np.concatenate([w2t_h, b2[None, :].astype(np_bf16)], axis=0)

    in_maps = []
    positions = []
    for c in range(N_CORES):
        lo, hi, gid, stok, dtok = cores[c]
        n = hi - lo
        order = np.argsort(gid, kind="stable")
        sorted_g = gid[order]
        gstart_sorted = np.searchsorted(sorted_g, np.arange(NG))
        rank_in_group = np.arange(n) - gstart_sorted[sorted_g]
        slot_sorted = starts[sorted_g] + rank_in_group
        pos = np.empty(n, dtype=np.int64)
        pos[order] = slot_sorted
        positions.append(pos)

        stok_s = np.zeros(e_slots, dtype=np.int16)
        dtok_s = np.zeros(e_slots, dtype=np.int16)
        stok_s[pos] = stok
        dtok_s[pos] = dtok
        idx_np = np.zeros((nm, 128, W // 8), dtype=np.int16)
        for m, spec in enumerate(macros):
            lm, mo = spec["L"], spec["off"]
            sw = stok_s[mo:mo + lm].reshape(lm // 16, 16).T
            dw = dtok_s[mo:mo + lm].reshape(lm // 16, 16).T
            blk = np.concatenate([sw, dw], axis=1)  # [16, lm/8]
            idx_np[m, :, :lm // 8] = np.tile(blk, (8, 1))

        eaef_h = np.zeros((2 * EA, e_slots), dtype=np_bf16)
        eaef_h[:, pos] = np.concatenate(
            [edge_attr[lo:hi], edge_f[lo:hi]], axis=1
        ).T.astype(np_bf16)

        in_maps.append({
            "xg": xgT,
            "w1t": w1t_h,
            "w2t": w2t_h,
            "idx": idx_np,
            "eaef": eaef_h,
        })

    meta = {"e_pc": e_pc, "n_edges": n_edges, "positions": positions}
    return nc, in_maps, meta


def kernel(x, edge_index, edge_attr, edge_f, w1, b1, w2, b2):
    global LAST_RESULTS
    nc, in_maps, meta = prepare(
        x, edge_index, edge_attr, edge_f, w1, b1, w2, b2
    )
    res = run_bass_kernel_spmd(nc, in_maps, list(range(N_CORES)), trace=TRACE)
    LAST_RESULTS = res

    parts = []
    for c in range(N_CORES):
        outT = np.asarray(res.results[c]["outT"]).astype(np.float32)
        parts.append(outT[:, meta["positions"][c]].T)
    return np.ascontiguousarray(np.concatenate(parts, axis=0), dtype=np.float32)
